# revision 7
# baseline (speedup 1.0000x reference)
"""Trainium2 Bass kernel for nn_Mnist_lmdSplineKAN.

Sharding: data-parallel over batch, 8 cores x 128 rows. All params replicated.

Per-core math (I=784 inputs, H=10 heads, O=64, 8 B-spline basis fns, order 3,
5 uniform intervals on [0,1)):
  x5 = min(5x, 4.9995); u = mod(x5, 1); masks m_t via is_ge chain
  pieces P_d(u): P0=w^3, P1=3u^3-6u^2+4, P2=3w^3-6w^2+4, P3=u^3 (w=1-u)
  basis feature f_{t+d} = m_t * P_d  (stored PR[s] = P_{3-s})
  features (8 basis + silu) quantized to fp8e4; weights fp8e4 with per-column
  (h,o) scales; main matmul TRANSPOSED (y[ho,b]) in 5 PSUM tiles of [128,128]
  via DoubleRow fp8 matmuls over chunk pairs (contraction 256/instr).
  tanh folds the fp8 column scales via ACT per-partition scale AP.
  Layer2: 5 matmuls contracting ho-slices + ones-row matmul for b1; tanh;
  W2-mult + reduce + b2 on DVE; out (128,10) f32.

I tiled as 6 chunks of 128 (3 DoubleRow pairs) + 1 tail chunk of 16 rows
(plain fp8 matmuls). Feature combine runs on DVE except j=4 (gpsimd).
fp8 conversion of f16 feature sums runs on ACT (single act table:
silu/tanh/square/copy all live in silu_and_others).
"""
import sys, types
import numpy as np

B, I, O, H, NB = 1024, 784, 64, 10, 8
NC = 8
BC = B // NC      # 128
CH = 7            # 6 full 128-row chunks + 1 of 16
PAIRS = 3
PLAST = 16
HO = H * O        # 640
HS = 5            # ho-slices of 128
D2 = H * 32       # 320
NF = NB + 1       # 9 features
FREE = CH * BC    # 896
X5MAX = 4.99951171875  # largest f16 below 5.0

# j processed on gpsimd
JGP = 4
# DVE combine order (feature-completion wavefront)
JORDER_DVE = (7, 0, 6, 1, 5, 2, 3)
# matmul emission order: j4 (gpsimd) lands before j3 (DVE last)
JORDER_MM = (7, 0, 6, 1, 5, 2, 4, 3, NB)   # NB == silu feature index


def _install_ntff_hook():
    if "antenv.axon_hooks" in sys.modules:
        return
    try:
        import antenv
        mod = types.ModuleType("antenv.axon_hooks")
        _h = [None]
        mod.set_axon_ntff_profile_hook = lambda h: _h.__setitem__(0, h)
        mod.get_axon_ntff_profile_hook = lambda: _h[0]
        sys.modules["antenv.axon_hooks"] = mod
        antenv.axon_hooks = mod
        from trn_agent_boot.trn_boot import _ntff_profile_via_ctypes
        h = _ntff_profile_via_ctypes("/opt/axon/libaxon_pjrt.so")
        if h is not None:
            mod.set_axon_ntff_profile_hook(h)
    except Exception:
        pass


_CACHE = {}


def _jparams(j):
    tlo = max(0, j - 3)
    k = min(4, j) - tlo + 1
    s0 = 3 - min(j, 3)
    return tlo, k, s0


def _build():
    if "nc" in _CACHE:
        return _CACHE["nc"]
    import concourse.bacc as bacc
    import concourse.bass as bass
    import concourse.tile as tile
    from concourse import mybir
    from contextlib import ExitStack

    f32, f16, f8 = mybir.dt.float32, mybir.dt.float16, mybir.dt.float8e4
    ALU = mybir.AluOpType
    AF = mybir.ActivationFunctionType
    DR = mybir.MatmulPerfMode.DoubleRow

    nc = bacc.Bacc("TRN2", target_bir_lowering=False, debug=False)
    x_d = nc.dram_tensor("x", (128, CH, BC), f32, kind="ExternalInput").ap()
    # pair weights: per j one piece [128, PAIRS, 2, HO] fp8, j-major flat
    wp_d = nc.dram_tensor("wp", (NF * 128 * PAIRS * 2 * HO,), f8,
                          kind="ExternalInput").ap()
    wt_d = nc.dram_tensor("wt", (PLAST, NF * HO), f8, kind="ExternalInput").ap()
    c16_d = nc.dram_tensor("c16", (128, HS * D2), f16, kind="ExternalInput").ap()
    b1_d = nc.dram_tensor("b1", (1, D2), f16, kind="ExternalInput").ap()
    # c32: [w2 bcast (320) | b2 bcast (10) | colscale (5)]
    c32_d = nc.dram_tensor("c32", (128, D2 + H + HS), f32,
                           kind="ExternalInput").ap()
    out_d = nc.dram_tensor("out", (BC, H), f32, kind="ExternalOutput").ap()

    with tile.TileContext(nc) as tc, ExitStack() as ctx:
        sb = ctx.enter_context(tc.tile_pool(name="sb", bufs=1))
        ps = ctx.enter_context(tc.tile_pool(name="ps", bufs=1, space="PSUM"))

        # ---- input DMAs: x first on the scalar HW queue ----
        xt = sb.tile([128, CH, BC], f32, tag="xt")
        nc.scalar.dma_start(xt[:], x_d)
        c16 = sb.tile([128, HS * D2], f16, tag="c16")
        nc.scalar.dma_start(c16[:], c16_d)
        w1t = c16[:].rearrange("p (k d) -> p k d", d=D2)
        b1r = sb.tile([1, D2], f16, tag="b1r")
        nc.scalar.dma_start(b1r[:], b1_d)
        c32 = sb.tile([128, D2 + H + HS], f32, tag="c32")
        nc.scalar.dma_start(c32[:], c32_d)
        w2b = c32[:, 0:D2]
        b2b = c32[:, D2:D2 + H]
        wtail = sb.tile([PLAST, NF * HO], f8, tag="wtail")
        nc.scalar.dma_start(wtail[:], wt_d)

        # ---- pair weights on the sync HW queue, wavefront order ----
        wg = {}
        PIECE = 128 * PAIRS * 2 * HO
        for j in JORDER_MM:
            t = sb.tile([128, PAIRS, 2, HO], f8, tag=f"wg{j}", name=f"wg{j}")
            src = bass.AP(tensor=wp_d.tensor, offset=j * PIECE,
                          ap=[[PAIRS * 2 * HO, 128], [1, PAIRS * 2 * HO]])
            nc.sync.dma_start(t[:], src)
            wg[j] = t

        ones = sb.tile([1, 128], f16, tag="ones")
        nc.vector.memset(ones[:], 1.0)

        xr = xt[:].rearrange("p c b -> p (c b)")

        def T(tag, dt=f16):
            return sb.tile([128, FREE], dt, tag=tag, name=tag)

        # ---- fp8 feature tiles ----
        fq = {}
        for j in range(NF):
            fq[j] = sb.tile([128, CH, BC], f8, tag=f"fq{j}", name=f"fq{j}")

        # silu feature straight to fp8 on ACT (loads the one act table)
        nc.scalar.activation(fq[NB][:].rearrange("p c b -> p (c b)"), xr,
                             AF.Silu)

        # ---- index math: ti = round(5x-0.5) (int conv), u = 5x - ti ----
        i32 = mybir.dt.int32
        ti32 = T("ti32", i32)
        nc.vector.tensor_scalar(ti32[:], xr, 5.0, -0.5, op0=ALU.mult,
                                op1=ALU.add)
        u = T("u")
        nc.vector.scalar_tensor_tensor(u[:], xr, 5.0, ti32[:], op0=ALU.mult,
                                       op1=ALU.subtract)
        tif = T("tif")
        nc.vector.tensor_copy(tif[:], ti32[:])
        M = sb.tile([128, 5, FREE], f16, tag="M")
        for t in range(5):
            nc.vector.tensor_scalar(M[:, t, :], tif[:], float(t), None,
                                    op0=ALU.is_equal)

        # ---- pieces: PR[s] = P_{3-s}; PR0=u^3 PR1=P2(w) PR2=P1(u) PR3=w^3 ----
        u2 = T("u2")
        nc.scalar.activation(u2[:], u[:], AF.Square)
        w2 = T("w2")
        nc.scalar.activation(w2[:], u[:], AF.Square, bias=1.0, scale=-1.0)
        w_ = T("w_")
        nc.scalar.activation(w_[:], u[:], AF.Copy, bias=1.0, scale=-1.0)
        PR = sb.tile([128, 4, FREE], f16, tag="PR")
        nc.vector.tensor_tensor(PR[:, 0, :], u[:], u2[:], op=ALU.mult)   # u^3
        nc.vector.tensor_tensor(PR[:, 3, :], w_[:], w2[:], op=ALU.mult)  # w^3
        rw = T("rw"); rw2 = T("rw2")
        nc.vector.tensor_scalar(rw[:], w2[:], -2.0, None, op0=ALU.mult)
        nc.vector.tensor_tensor(rw2[:], rw[:], PR[:, 3, :], op=ALU.add)
        nc.vector.tensor_scalar(PR[:, 1, :], rw2[:], 3.0, 4.0, op0=ALU.mult,
                                op1=ALU.add)                              # P2
        ru = T("ru"); ru2 = T("ru2")
        nc.vector.tensor_scalar(ru[:], u2[:], -2.0, None, op0=ALU.mult)
        nc.vector.tensor_tensor(ru2[:], ru[:], PR[:, 0, :], op=ALU.add)
        nc.vector.tensor_scalar(PR[:, 2, :], ru2[:], 3.0, 4.0, op0=ALU.mult,
                                op1=ALU.add)                              # P1

        # ---- combine: f_j = sum_t m_t * PR[s0+t-tlo]; j=4 on gpsimd ----
        fs = {}
        for j in range(NB):
            fs[j] = T(f"fs{j}")
        tk = sb.tile([128, 4, FREE], f16, tag="tk")
        t2 = sb.tile([128, 2, FREE], f16, tag="t2")
        tkg = sb.tile([128, 4, FREE], f16, tag="tkg")
        t2g = sb.tile([128, 2, FREE], f16, tag="t2g")

        def combine(eng, j, tkb, t2b):
            tlo, k, s0 = _jparams(j)
            if k == 1:
                eng.tensor_tensor(fs[j][:], M[:, tlo, :], PR[:, s0, :],
                                  op=ALU.mult)
                return
            eng.tensor_tensor(tkb[:, 0:k, :], M[:, tlo:tlo + k, :],
                              PR[:, s0:s0 + k, :], op=ALU.mult)
            if k == 2:
                eng.tensor_tensor(fs[j][:], tkb[:, 0, :], tkb[:, 1, :],
                                  op=ALU.add)
            elif k == 3:
                eng.tensor_tensor(t2b[:, 0, :], tkb[:, 0, :], tkb[:, 1, :],
                                  op=ALU.add)
                eng.tensor_tensor(fs[j][:], t2b[:, 0, :], tkb[:, 2, :],
                                  op=ALU.add)
            else:
                eng.tensor_tensor(t2b[:], tkb[:, 0:2, :], tkb[:, 2:4, :],
                                  op=ALU.add)
                eng.tensor_tensor(fs[j][:], t2b[:, 0, :], t2b[:, 1, :],
                                  op=ALU.add)

        # gpsimd: j=4 fully (mults + adds), fp8 final written by ACT below
        combine(nc.gpsimd, JGP, tkg, t2g)
        for j in JORDER_DVE:
            combine(nc.vector, j, tk, t2)

        # fp8 conversion on ACT in completion order
        for j in (7, 0, 6, 1, 5, 2, 4, 3):
            nc.scalar.activation(fq[j][:].rearrange("p c b -> p (c b)"),
                                 fs[j][:], AF.Copy)

        # ---- main matmuls: transposed, DoubleRow fp8 over chunk pairs ----
        psum = [ps.tile([128, BC], f32, tag=f"y{hs}", name=f"y{hs}")
                for hs in range(HS)]
        nj = len(JORDER_MM)
        for jn, j in enumerate(JORDER_MM):
            first, last = (jn == 0), (jn == nj - 1)
            for p in range(PAIRS):
                for hs in range(HS):
                    nc.tensor.matmul(
                        psum[hs][:],
                        wg[j][:, p, :, hs * 128:(hs + 1) * 128],
                        fq[j][:, 2 * p:2 * p + 2, :],
                        start=(first and p == 0), stop=False,
                        perf_mode=DR)
            for hs in range(HS):
                nc.tensor.matmul(
                    psum[hs][:],
                    wtail[:, j * HO + hs * 128:j * HO + (hs + 1) * 128],
                    fq[j][0:PLAST, 6, :],
                    start=False, stop=last)

        # ---- tail: h1 = tanh(y * colscale) per slice, layer2, reduce ----
        h1 = []
        for hs in range(HS):
            t = sb.tile([128, BC], f16, tag=f"h1{hs}", name=f"h1{hs}")
            nc.scalar.activation(t[:], psum[hs][:], AF.Tanh,
                                 scale=c32[:, D2 + H + hs:D2 + H + hs + 1])
            h1.append(t)
        ps2 = ps.tile([128, D2], f32, tag="ps2")
        for hs in range(HS):
            nc.tensor.matmul(ps2[:], h1[hs][:], w1t[:, hs, :],
                             start=(hs == 0), stop=False)
        nc.tensor.matmul(ps2[:], ones[:], b1r[:], start=False, stop=True)
        h2 = sb.tile([128, D2], f32, tag="h2")
        nc.scalar.activation(h2[:], ps2[:], AF.Tanh)
        prod = sb.tile([128, D2], f32, tag="prod")
        nc.vector.tensor_tensor(prod[:], h2[:], w2b, op=ALU.mult)
        red = sb.tile([128, H], f32, tag="red")
        nc.vector.tensor_reduce(red[:], prod[:].rearrange("p (h d) -> p h d",
                                                          d=32),
                                axis=mybir.AxisListType.X, op=ALU.add)
        lg = sb.tile([128, H], f32, tag="lg")
        nc.vector.tensor_tensor(lg[:], red[:], b2b, op=ALU.add)
        nc.sync.dma_start(out_d, lg[:])

    nc.compile()
    _CACHE["nc"] = nc
    return nc


def _prep_inputs(x, coef, scale_base, scale_sp, lmd, W1, b1, W2, b2):
    import ml_dtypes
    E4 = ml_dtypes.float8_e4m3   # TRN2 fp8e4: IEEE e4m3, max normal 240
    xf = np.asarray(x, np.float64).reshape(B, I)
    coef = np.asarray(coef, np.float64)
    eff = coef * np.asarray(scale_sp, np.float64)[..., None] \
        * np.asarray(lmd, np.float64)[:, :, None, None] / 6.0
    sbl = np.asarray(scale_base, np.float64) \
        * np.asarray(lmd, np.float64)[:, :, None]
    wbig = np.concatenate([eff, sbl[..., None]], -1)           # (H,I,O,9)
    wi = np.ascontiguousarray(wbig.transpose(1, 3, 0, 2)).reshape(I, NF, HO)
    # per-(h,o)-column fp8 scale over (i, j)
    s_col = np.abs(wi).max(axis=(0, 1)) / 240.0 * 1.05         # (640,)
    s_col[s_col == 0] = 1.0
    wq = np.asarray(wi / s_col[None, None, :], E4)             # (I, NF, HO)

    # pair pieces, j-major: [j][r][p][k][ho] = wq[p*256 + k*128 + r, j, ho]
    wq64 = wq.view(np.uint8)
    wp = np.empty((NF, 128, PAIRS, 2, HO), np.uint8)
    for p in range(PAIRS):
        for k in range(2):
            blk = wq64[p * 256 + k * 128: p * 256 + (k + 1) * 128]  # (128,NF,HO)
            wp[:, :, p, k, :] = blk.transpose(1, 0, 2)
    wp = wp.reshape(-1).view(E4)
    wt = np.ascontiguousarray(wq64[768:I].reshape(PLAST, NF * HO)).view(E4)

    W1 = np.asarray(W1, np.float64)
    w1bd = np.zeros((HO, D2))
    for h in range(H):
        w1bd[h * O:(h + 1) * O, h * 32:(h + 1) * 32] = W1[h]
    c16 = np.ascontiguousarray(
        w1bd.reshape(HS, 128, D2).transpose(1, 0, 2)).astype(
            np.float16).reshape(128, HS * D2)
    b1c = np.asarray(b1, np.float16).reshape(1, D2).copy()
    sc = np.ascontiguousarray(s_col.reshape(HS, 128).T)        # (128, 5)
    c32 = np.ascontiguousarray(np.concatenate([
        np.broadcast_to(np.asarray(W2, np.float32).reshape(D2), (128, D2)),
        np.broadcast_to(np.asarray(b2, np.float32).reshape(H), (128, H)),
        sc.astype(np.float32)], 1).astype(np.float32))

    in_maps = []
    for core in range(NC):
        xs = xf[core * BC:(core + 1) * BC].T                   # (784,128)
        xdev = np.zeros((128, CH, BC), np.float32)
        for c in range(CH):
            rows = xs[c * 128:min((c + 1) * 128, I)]
            xdev[0:rows.shape[0], c, :] = rows
        in_maps.append({"x": xdev, "wp": wp, "wt": wt, "c16": c16,
                        "b1": b1c, "c32": c32})
    return in_maps


def run(inputs, trace=False, tmpdir=None):
    _install_ntff_hook()
    from concourse.bass_utils import run_bass_kernel_spmd
    nc = _build()
    in_maps = _prep_inputs(**inputs)
    res = run_bass_kernel_spmd(nc, in_maps, core_ids=list(range(NC)),
                               trace=trace, tmpdir=tmpdir)
    out = np.concatenate([r["out"] for r in res.results], 0)
    return out.astype(np.float32), res


def kernel(**inputs):
    out, _ = run(inputs)
    return out


# revision 11
# speedup vs baseline: 1.1428x; 1.1428x over previous
"""Trainium2 Bass kernel for nn_Mnist_lmdSplineKAN.

Sharding: data-parallel over batch, 8 cores x 128 rows. All params replicated.

Per-core math (I=784, H=10, O=64, 8 cubic B-spline basis fns on 5 intervals):
  ti = round(5x-0.5); u = 5x - ti; masks m_t = (ti == t), t = 0..4
  pieces PR[s]: PR0=u^3, PR1=3u^3-6u^2+4, PR2=3w^3-6w^2+4, PR3=w^3 (w=1-u)
  basis feature f_j = sum_t m_t * PR[3-j+t]; masks are disjoint, so each
  masked product m_t*PR[s] is fed to the PE as its OWN feature (slot) and
  PSUM accumulation performs the sum -- no DVE adds.  Product slot (t,s)
  contracts against the basis-j weight tile with j = t+3-s, so weights stay
  in the 9-feature basis (4.4 MB fp8), each tile read 4x by the PE.

  Features and weights are fp8e4 (IEEE e4m3, max 240); weights carry
  per-(h,o)-column scales, descaled on the PSUM->SBUF copy.  Main matmuls
  are DoubleRow fp8 over chunk pairs (contraction 256/instr), features
  stationary.  I tiled as 6 chunks of 128 (3 pairs) + 16-row tail; tail
  products are gathered into [128,128] tiles by small SBUF DMAs and hit
  with plain fp8 matmuls.

  Tail: ysb = y*colscale (DVE f32), transpose 5x128x128 (PE, f32 identity),
  h1T = tanh (ACT), layer2 = 5 matmuls over ho-slices + ones-row for b1,
  h2 = tanh, logits = rowsum(h2*W2bcast) + b2 (DVE), out (128,10) f32.

DMA: x first on the sync HW queue; weights on the gpsimd SWDGE queue
(9 j-pieces in first-use order); consts + tail weights on the scalar HW
queue; product gathers + output on sync.
"""
import sys, types
import numpy as np

B, I, O, H, NB = 1024, 784, 64, 10, 8
NC = 8
BC = B // NC      # 128
CH = 7            # 6 full 128-row chunks + 1 of 16
PAIRS = 3
PLAST = 16
HO = H * O        # 640
HS = 5
D2 = H * 32       # 320
NF = NB + 1       # 9 weight features (8 basis + silu)
FREE = CH * BC    # 896

# weight-piece DMA order = first PE use order (t asc, s asc -> j = t+3-s)
JORDER_W = (3, 2, 1, 0, 4, 5, 6, 7, 8)


def _install_ntff_hook():
    if "antenv.axon_hooks" in sys.modules:
        return
    try:
        import antenv
        mod = types.ModuleType("antenv.axon_hooks")
        _h = [None]
        mod.set_axon_ntff_profile_hook = lambda h: _h.__setitem__(0, h)
        mod.get_axon_ntff_profile_hook = lambda: _h[0]
        sys.modules["antenv.axon_hooks"] = mod
        antenv.axon_hooks = mod
        from trn_agent_boot.trn_boot import _ntff_profile_via_ctypes
        h = _ntff_profile_via_ctypes("/opt/axon/libaxon_pjrt.so")
        if h is not None:
            mod.set_axon_ntff_profile_hook(h)
    except Exception:
        pass


_CACHE = {}


def _build():
    if "nc" in _CACHE:
        return _CACHE["nc"]
    import concourse.bacc as bacc
    import concourse.bass as bass
    import concourse.tile as tile
    from concourse import mybir
    from contextlib import ExitStack

    f32, f16, f8 = mybir.dt.float32, mybir.dt.float16, mybir.dt.float8e4
    i32 = mybir.dt.int32
    ALU = mybir.AluOpType
    AF = mybir.ActivationFunctionType
    DR = mybir.MatmulPerfMode.DoubleRow

    nc = bacc.Bacc("TRN2", target_bir_lowering=False, debug=False)
    x_d = nc.dram_tensor("x", (128, CH, BC), f32, kind="ExternalInput").ap()
    wp_d = nc.dram_tensor("wp", (NF * 128 * PAIRS * 2 * HO,), f8,
                          kind="ExternalInput").ap()
    # tail weights: 21 slots (20 products + silu) x 16 rows
    wt_d = nc.dram_tensor("wt", (336, HO), f8, kind="ExternalInput").ap()
    c16_d = nc.dram_tensor("c16", (128, HS * D2), f16, kind="ExternalInput").ap()
    b1_d = nc.dram_tensor("b1", (1, D2), f16, kind="ExternalInput").ap()
    # c32: [w2 bcast 320 | b2 bcast 10 | colscale bcast 640]
    c32_d = nc.dram_tensor("c32", (128, D2 + H + HO), f32,
                           kind="ExternalInput").ap()
    idt_d = nc.dram_tensor("idt", (128, 128), f32, kind="ExternalInput").ap()
    out_d = nc.dram_tensor("out", (BC, H), f32, kind="ExternalOutput").ap()

    with tile.TileContext(nc) as tc, ExitStack() as ctx:
        sb = ctx.enter_context(tc.tile_pool(name="sb", bufs=1))
        ps = ctx.enter_context(tc.tile_pool(name="ps", bufs=1, space="PSUM"))

        # ---- x first, on the sync HW queue ----
        xt = sb.tile([128, CH, BC], f32, tag="xt")
        nc.sync.dma_start(xt[:], x_d)

        # ---- weights on the gpsimd SWDGE queue, first-use order ----
        wg = {}
        PIECE = 128 * PAIRS * 2 * HO
        for j in JORDER_W:
            t = sb.tile([128, PAIRS, 2, HO], f8, tag=f"wg{j}", name=f"wg{j}")
            src = bass.AP(tensor=wp_d.tensor, offset=j * PIECE,
                          ap=[[PAIRS * 2 * HO, 128], [1, PAIRS * 2 * HO]])
            nc.gpsimd.dma_start(t[:], src)
            wg[j] = t

        # ---- consts on the scalar HW queue ----
        c16 = sb.tile([128, HS * D2], f16, tag="c16")
        nc.scalar.dma_start(c16[:], c16_d)
        w1t = c16[:].rearrange("p (k d) -> p k d", d=D2)
        b1r = sb.tile([1, D2], f16, tag="b1r")
        nc.scalar.dma_start(b1r[:], b1_d)
        c32 = sb.tile([128, D2 + H + HO], f32, tag="c32")
        nc.scalar.dma_start(c32[:], c32_d)
        w2b = c32[:, 0:D2]
        b2b = c32[:, D2:D2 + H]
        sbc = c32[:, D2 + H:]
        idt = sb.tile([128, 128], f32, tag="idt")
        nc.scalar.dma_start(idt[:], idt_d)
        wt0 = sb.tile([128, HO], f8, tag="wt0")
        nc.scalar.dma_start(wt0[:], wt_d[0:128, :])
        wt1 = sb.tile([128, HO], f8, tag="wt1")
        nc.scalar.dma_start(wt1[:], wt_d[128:256, :])
        wt2 = sb.tile([80, HO], f8, tag="wt2")
        nc.scalar.dma_start(wt2[:], wt_d[256:336, :])

        ones = sb.tile([1, 128], f16, tag="ones")
        nc.vector.memset(ones[:], 1.0)

        xr = xt[:].rearrange("p c b -> p (c b)")

        def T(tag, dt=f16):
            return sb.tile([128, FREE], dt, tag=tag, name=tag)

        # silu feature straight to fp8 on ACT (loads the act table; only
        # silu/square/copy/tanh used => single table)
        fqs = sb.tile([128, CH, BC], f8, tag="fqs")
        nc.scalar.activation(fqs[:].rearrange("p c b -> p (c b)"), xr, AF.Silu)

        # ---- index math on DVE ----
        ti32 = T("ti32", i32)
        nc.vector.tensor_scalar(ti32[:], xr, 5.0, -0.5, op0=ALU.mult,
                                op1=ALU.add)
        u = T("u")
        nc.vector.scalar_tensor_tensor(u[:], xr, 5.0, ti32[:], op0=ALU.mult,
                                       op1=ALU.subtract)
        tif = T("tif")
        nc.vector.tensor_copy(tif[:], ti32[:])
        M = sb.tile([128, 5, FREE], f16, tag="M")
        for t in range(5):
            nc.vector.tensor_scalar(M[:, t, :], tif[:], float(t), None,
                                    op0=ALU.is_equal)

        # ---- pieces ----
        u2 = T("u2")
        nc.scalar.activation(u2[:], u[:], AF.Square)
        w2 = T("w2")
        nc.scalar.activation(w2[:], u[:], AF.Square, bias=1.0, scale=-1.0)
        w_ = T("w_")
        nc.vector.tensor_scalar(w_[:], u[:], -1.0, 1.0, op0=ALU.mult,
                                op1=ALU.add)
        PR = sb.tile([128, 4, FREE], f16, tag="PR")
        nc.vector.tensor_tensor(PR[:, 0, :], u[:], u2[:], op=ALU.mult)   # u^3
        nc.vector.tensor_tensor(PR[:, 3, :], w_[:], w2[:], op=ALU.mult)  # w^3
        rw = T("rw"); rw2 = T("rw2")
        nc.vector.tensor_scalar(rw[:], w2[:], -2.0, None, op0=ALU.mult)
        nc.vector.tensor_tensor(rw2[:], rw[:], PR[:, 3, :], op=ALU.add)
        nc.vector.tensor_scalar(PR[:, 1, :], rw2[:], 3.0, 4.0, op0=ALU.mult,
                                op1=ALU.add)                     # 3w^3-6w^2+4
        ru = T("ru"); ru2 = T("ru2")
        nc.vector.tensor_scalar(ru[:], u2[:], -2.0, None, op0=ALU.mult)
        nc.vector.tensor_tensor(ru2[:], ru[:], PR[:, 0, :], op=ALU.add)
        nc.vector.tensor_scalar(PR[:, 2, :], ru2[:], 3.0, 4.0, op0=ALU.mult,
                                op1=ALU.add)                     # 3u^3-6u^2+4

        # ---- masked products: per t one broadcast TT (f16), ACT -> fp8 ----
        GS = {}
        FQ = {}
        for t in range(5):
            GS[t] = sb.tile([128, 4, FREE], f16, tag=f"GS{t}", name=f"GS{t}")
            FQ[t] = sb.tile([128, 4, CH, BC], f8, tag=f"FQ{t}", name=f"FQ{t}")
        for t in range(5):
            mslice = M[:, t, :]
            mb = bass.AP(tensor=mslice.tensor, offset=mslice.offset,
                         ap=[list(mslice.ap[0]), [0, 4], [1, FREE]])
            nc.vector.tensor_tensor(GS[t][:], mb, PR[:], op=ALU.mult)
        for t in range(5):
            nc.scalar.activation(FQ[t][:].rearrange("p s c b -> p (s c b)"),
                                 GS[t][:].rearrange("p s f -> p (s f)"),
                                 AF.Copy)

        # ---- main matmuls: DoubleRow fp8 over chunk pairs ----
        # product slot (t,s) -> weight j = t+3-s; PR[s]: s maps via piece
        # index: PR[0]=u^3 pairs with j=t+3 ... PR[3]=w^3 with j=t.
        psum = [ps.tile([128, D2], f32, tag=f"y{nh}", name=f"y{nh}")
                for nh in range(2)]
        first = True
        for t in range(5):
            for s in range(4):
                j = t + 3 - s
                for p in range(PAIRS):
                    for nh in range(2):
                        nc.tensor.matmul(
                            psum[nh][:],
                            FQ[t][:, s, 2 * p:2 * p + 2, :],
                            wg[j][:, p, :, nh * D2:(nh + 1) * D2],
                            start=first and p == 0, stop=False,
                            perf_mode=DR)
                first = False
        for p in range(PAIRS):
            for nh in range(2):
                nc.tensor.matmul(psum[nh][:], fqs[:, 2 * p:2 * p + 2, :],
                                 wg[8][:, p, :, nh * D2:(nh + 1) * D2],
                                 start=False, stop=False, perf_mode=DR)

        # ---- tail chunk: gather product slots to [128,128] fp8 tiles ----
        # slots 0..7 = (t0,s0..3),(t1,*); 8..15 = (t2,*),(t3,*); 16..19=(t4,*)
        gath = [sb.tile([128, BC], f8, tag=f"ga{g}", name=f"ga{g}")
                for g in range(2)]
        gath2 = sb.tile([80, BC], f8, tag="ga2")
        for g in range(2):
            for i_t in range(2):
                t = 2 * g + i_t
                nc.sync.dma_start(gath[g][64 * i_t:64 * i_t + 64, :],
                                  FQ[t][0:PLAST, :, 6, :])
        nc.sync.dma_start(gath2[0:64, :], FQ[4][0:PLAST, :, 6, :])
        nc.sync.dma_start(gath2[64:80, :], fqs[0:PLAST, 6, :])
        for nh in range(2):
            nc.tensor.matmul(psum[nh][:], gath[0][:],
                             wt0[:, nh * D2:(nh + 1) * D2],
                             start=False, stop=False)
            nc.tensor.matmul(psum[nh][:], gath[1][:],
                             wt1[:, nh * D2:(nh + 1) * D2],
                             start=False, stop=False)
            nc.tensor.matmul(psum[nh][:], gath2[:],
                             wt2[:, nh * D2:(nh + 1) * D2],
                             start=False, stop=True)

        # ---- tail: descale, transpose, tanh, layer2, reduce ----
        ysb = sb.tile([128, HO], f32, tag="ysb")
        for nh in range(2):
            nc.vector.tensor_tensor(ysb[:, nh * D2:(nh + 1) * D2],
                                    psum[nh][:],
                                    sbc[:, nh * D2:(nh + 1) * D2],
                                    op=ALU.mult)
        h1t = []
        for k in range(HS):
            pt = ps.tile([128, 128], f32, tag=f"pt{k}", name=f"pt{k}")
            nc.tensor.transpose(pt[:], ysb[:, k * 128:(k + 1) * 128], idt[:])
            st = sb.tile([128, 128], f16, tag=f"h1t{k}", name=f"h1t{k}")
            nc.scalar.activation(st[:], pt[:], AF.Tanh)
            h1t.append(st)
        ps2 = ps.tile([128, D2], f32, tag="ps2")
        for k in range(HS):
            nc.tensor.matmul(ps2[:], h1t[k][:], w1t[:, k, :],
                             start=(k == 0), stop=False)
        nc.tensor.matmul(ps2[:], ones[:], b1r[:], start=False, stop=True)
        h2 = sb.tile([128, D2], f32, tag="h2")
        nc.scalar.activation(h2[:], ps2[:], AF.Tanh)
        prod = sb.tile([128, D2], f32, tag="prod")
        nc.vector.tensor_tensor(prod[:], h2[:], w2b, op=ALU.mult)
        red = sb.tile([128, H], f32, tag="red")
        nc.vector.tensor_reduce(red[:], prod[:].rearrange("p (h d) -> p h d",
                                                          d=32),
                                axis=mybir.AxisListType.X, op=ALU.add)
        lg = sb.tile([128, H], f32, tag="lg")
        nc.vector.tensor_tensor(lg[:], red[:], b2b, op=ALU.add)
        nc.sync.dma_start(out_d, lg[:])

    nc.compile()
    _CACHE["nc"] = nc
    return nc


def _prep_inputs(x, coef, scale_base, scale_sp, lmd, W1, b1, W2, b2):
    import ml_dtypes
    E4 = ml_dtypes.float8_e4m3   # TRN2 fp8e4: IEEE e4m3, max normal 240
    xf = np.asarray(x, np.float64).reshape(B, I)
    coef = np.asarray(coef, np.float64)
    eff = coef * np.asarray(scale_sp, np.float64)[..., None] \
        * np.asarray(lmd, np.float64)[:, :, None, None] / 6.0
    sbl = np.asarray(scale_base, np.float64) \
        * np.asarray(lmd, np.float64)[:, :, None]
    wbig = np.concatenate([eff, sbl[..., None]], -1)           # (H,I,O,9)
    wi = np.ascontiguousarray(wbig.transpose(1, 3, 0, 2)).reshape(I, NF, HO)
    s_col = np.abs(wi).max(axis=(0, 1)) / 240.0 * 1.05         # (640,)
    s_col[s_col == 0] = 1.0
    wq = np.asarray(wi / s_col[None, None, :], E4)             # (I, NF, HO)

    # pair pieces, j-major: [j][r][p][k][ho] = wq[p*256 + k*128 + r, j, ho]
    wq8 = wq.view(np.uint8)
    wp = np.empty((NF, 128, PAIRS, 2, HO), np.uint8)
    for p in range(PAIRS):
        for k in range(2):
            blk = wq8[p * 256 + k * 128: p * 256 + (k + 1) * 128]
            wp[:, :, p, k, :] = blk.transpose(1, 0, 2)
    wp = wp.reshape(-1).view(E4)
    # tail rows follow the gather DMA order: within each t-block of 64 rows,
    # row = k*4 + s (k = tail I-row 0..15, s = piece slot); j = t+3-s
    wt = np.empty((336, HO), np.uint8)
    for t in range(5):
        base = t * 64
        for k in range(PLAST):
            for s in range(4):
                wt[base + k * 4 + s, :] = wq8[768 + k, t + 3 - s, :]
    wt[320:336, :] = wq8[768:I, 8, :]
    wt = wt.view(E4)

    W1 = np.asarray(W1, np.float64)
    w1bd = np.zeros((HO, D2))
    for h in range(H):
        w1bd[h * O:(h + 1) * O, h * 32:(h + 1) * 32] = W1[h]
    c16 = np.ascontiguousarray(
        w1bd.reshape(HS, 128, D2).transpose(1, 0, 2)).astype(
            np.float16).reshape(128, HS * D2)
    b1c = np.asarray(b1, np.float16).reshape(1, D2).copy()
    c32 = np.ascontiguousarray(np.concatenate([
        np.broadcast_to(np.asarray(W2, np.float32).reshape(D2), (128, D2)),
        np.broadcast_to(np.asarray(b2, np.float32).reshape(H), (128, H)),
        np.broadcast_to(s_col.astype(np.float32), (128, HO))],
        1).astype(np.float32))
    idt = np.eye(128, dtype=np.float32)

    in_maps = []
    for core in range(NC):
        xs = xf[core * BC:(core + 1) * BC].T                   # (784,128)
        xdev = np.zeros((128, CH, BC), np.float32)
        for c in range(CH):
            rows = xs[c * 128:min((c + 1) * 128, I)]
            xdev[0:rows.shape[0], c, :] = rows
        in_maps.append({"x": xdev, "wp": wp, "wt": wt, "c16": c16,
                        "b1": b1c, "c32": c32, "idt": idt})
    return in_maps


def run(inputs, trace=False, tmpdir=None):
    _install_ntff_hook()
    from concourse.bass_utils import run_bass_kernel_spmd
    nc = _build()
    in_maps = _prep_inputs(**inputs)
    res = run_bass_kernel_spmd(nc, in_maps, core_ids=list(range(NC)),
                               trace=trace, tmpdir=tmpdir)
    out = np.concatenate([r["out"] for r in res.results], 0)
    return out.astype(np.float32), res


def kernel(**inputs):
    out, _ = run(inputs)
    return out


# revision 12
# speedup vs baseline: 1.2207x; 1.0682x over previous
"""Trainium2 Bass kernel for nn_Mnist_lmdSplineKAN.

Sharding: data-parallel over batch, 8 cores x 128 rows. All params replicated.

Per-core math (I=784, H=10, O=64, 8 cubic B-spline basis fns on 5 intervals):
  ti = round(5x-0.5); u = 5x - ti; masks m_t = (ti == t)
  pieces PR[s]: PR0=u^3, PR1=3w^3-6w^2+4, PR2=3u^3-6u^2+4, PR3=w^3 (w=1-u)
  basis f_j = sum_t m_t * PR[3-j+t]; masks disjoint => each masked product
  is a valid standalone PE feature.  11 feature tiles are fed to the PE:
    f0=(t0,s3)->j0  f1=(t0,s2)->j1  f2=(t1,s3)->j1   [single products]
    f3=j2 f4=j3 f5=j4 f6=j5                          [DVE-fused sums]
    f7=(t3,s0)->j6  f8=(t4,s1)->j6  f9=(t4,s0)->j7   [single products]
    f10=silu                                         -> j8
  Each contracts against the 9-basis weight tile wg[j] (4.4 MB fp8 total).
  Products come from 5 batched broadcast TTs (GS[t] = m_t * PR[0:4]).

  fp8e4 everywhere (IEEE e4m3, max 240); per-(h,o)-column weight scales,
  descaled on the PSUM->SBUF copy.  Main matmuls: DoubleRow fp8 over chunk
  pairs, features stationary, one explicit LDWEIGHTS shared by the two
  output-half matmuls.  I = 6 chunks of 128 (3 pairs) + 16-row tail; tail
  rows of all 11 features are gathered into 2 tiles and hit with plain
  fp8 matmuls.

  Tail: ysb = y*colscale (DVE), 5x f32 transposes (PE), h1T = tanh (ACT),
  layer2 = 5 matmuls + ones-row for b1, h2 = tanh, logits = rowsum(h2*W2)
  + b2 (DVE), out (128,10) f32.

DMA: x on sync HW queue first; weights on gpsimd SWDGE (9 j-pieces in
first-use order); consts + tail weights on scalar; gathers + out on sync.
"""
import sys, types
import numpy as np

B, I, O, H, NB = 1024, 784, 64, 10, 8
NC = 8
BC = B // NC      # 128
CH = 7
PAIRS = 3
PLAST = 16
HO = H * O        # 640
HS = 5
D2 = H * 32       # 320
NF = NB + 1       # 9 weight features
FREE = CH * BC    # 896
NFEAT = 11

# feature table: (kind, payload)
#  ('p', (t, s))   single product slot -> j = t+3-s
#  ('f', j)        fused basis j (DVE adds)
#  ('s', None)     silu
FEATS = [('p', (0, 3)), ('p', (0, 2)), ('p', (1, 3)),
         ('f', 2), ('f', 3), ('f', 4), ('f', 5),
         ('p', (3, 0)), ('p', (4, 1)), ('p', (4, 0)), ('s', None)]


def _feat_j(f):
    kind, pl = FEATS[f]
    if kind == 'p':
        t, s = pl
        return t + 3 - s
    if kind == 'f':
        return pl
    return 8


# weight DMA order = first PE use order
JORDER_W = (0, 1, 2, 3, 4, 5, 6, 7, 8)


def _install_ntff_hook():
    if "antenv.axon_hooks" in sys.modules:
        return
    try:
        import antenv
        mod = types.ModuleType("antenv.axon_hooks")
        _h = [None]
        mod.set_axon_ntff_profile_hook = lambda h: _h.__setitem__(0, h)
        mod.get_axon_ntff_profile_hook = lambda: _h[0]
        sys.modules["antenv.axon_hooks"] = mod
        antenv.axon_hooks = mod
        from trn_agent_boot.trn_boot import _ntff_profile_via_ctypes
        h = _ntff_profile_via_ctypes("/opt/axon/libaxon_pjrt.so")
        if h is not None:
            mod.set_axon_ntff_profile_hook(h)
    except Exception:
        pass


_CACHE = {}


def _build():
    if "nc" in _CACHE:
        return _CACHE["nc"]
    import concourse.bacc as bacc
    import concourse.bass as bass
    import concourse.tile as tile
    from concourse import mybir
    from contextlib import ExitStack

    f32, f16, f8 = mybir.dt.float32, mybir.dt.float16, mybir.dt.float8e4
    i32 = mybir.dt.int32
    ALU = mybir.AluOpType
    AF = mybir.ActivationFunctionType
    DR = mybir.MatmulPerfMode.DoubleRow

    nc = bacc.Bacc("TRN2", target_bir_lowering=False, debug=False)
    x_d = nc.dram_tensor("x", (128, CH, BC), f32, kind="ExternalInput").ap()
    wp_d = nc.dram_tensor("wp", (NF * 128 * PAIRS * 2 * HO,), f8,
                          kind="ExternalInput").ap()
    # tail weights: 11 slots x 16 rows = 176 rows
    wt_d = nc.dram_tensor("wt", (NFEAT * PLAST, HO), f8,
                          kind="ExternalInput").ap()
    c16_d = nc.dram_tensor("c16", (128, HS * D2), f16, kind="ExternalInput").ap()
    b1_d = nc.dram_tensor("b1", (1, D2), f16, kind="ExternalInput").ap()
    c32_d = nc.dram_tensor("c32", (128, D2 + H + HO), f32,
                           kind="ExternalInput").ap()
    idt_d = nc.dram_tensor("idt", (128, 128), f32, kind="ExternalInput").ap()
    out_d = nc.dram_tensor("out", (BC, H), f32, kind="ExternalOutput").ap()

    with tile.TileContext(nc) as tc, ExitStack() as ctx:
        sb = ctx.enter_context(tc.tile_pool(name="sb", bufs=1))
        ps = ctx.enter_context(tc.tile_pool(name="ps", bufs=1, space="PSUM"))

        # ---- x on the sync HW queue, first ----
        xt = sb.tile([128, CH, BC], f32, tag="xt")
        nc.sync.dma_start(xt[:], x_d)

        # ---- weights on gpsimd SWDGE ----
        wg = {}
        PIECE = 128 * PAIRS * 2 * HO
        for j in JORDER_W:
            t = sb.tile([128, PAIRS, 2, HO], f8, tag=f"wg{j}", name=f"wg{j}")
            src = bass.AP(tensor=wp_d.tensor, offset=j * PIECE,
                          ap=[[PAIRS * 2 * HO, 128], [1, PAIRS * 2 * HO]])
            nc.gpsimd.dma_start(t[:], src)
            wg[j] = t

        # ---- consts on the scalar HW queue ----
        c16 = sb.tile([128, HS * D2], f16, tag="c16")
        nc.scalar.dma_start(c16[:], c16_d)
        w1t = c16[:].rearrange("p (k d) -> p k d", d=D2)
        b1r = sb.tile([1, D2], f16, tag="b1r")
        nc.scalar.dma_start(b1r[:], b1_d)
        c32 = sb.tile([128, D2 + H + HO], f32, tag="c32")
        nc.scalar.dma_start(c32[:], c32_d)
        w2b = c32[:, 0:D2]
        b2b = c32[:, D2:D2 + H]
        sbc = c32[:, D2 + H:]
        idt = sb.tile([128, 128], f32, tag="idt")
        nc.scalar.dma_start(idt[:], idt_d)
        wt0 = sb.tile([128, HO], f8, tag="wt0")
        nc.scalar.dma_start(wt0[:], wt_d[0:128, :])
        wt1 = sb.tile([NFEAT * PLAST - 128, HO], f8, tag="wt1")
        nc.scalar.dma_start(wt1[:], wt_d[128:NFEAT * PLAST, :])

        ones = sb.tile([1, 128], f16, tag="ones")
        nc.vector.memset(ones[:], 1.0)

        xr = xt[:].rearrange("p c b -> p (c b)")

        def T(tag, dt=f16):
            return sb.tile([128, FREE], dt, tag=tag, name=tag)

        # ---- index math on DVE ----
        ti32 = T("ti32", i32)
        nc.vector.tensor_scalar(ti32[:], xr, 5.0, -0.5, op0=ALU.mult,
                                op1=ALU.add)
        u = T("u")
        nc.vector.scalar_tensor_tensor(u[:], xr, 5.0, ti32[:], op0=ALU.mult,
                                       op1=ALU.subtract)
        # ACT kicks off u2/w2 as soon as u is ready (silu comes later)
        u2 = T("u2")
        nc.scalar.activation(u2[:], u[:], AF.Square)
        w2 = T("w2")
        nc.scalar.activation(w2[:], u[:], AF.Square, bias=1.0, scale=-1.0)
        # DVE hides the ACT round-trip behind masks
        tif = T("tif")
        nc.vector.tensor_copy(tif[:], ti32[:])
        M = sb.tile([128, 5, FREE], f16, tag="M")
        for t in range(5):
            nc.vector.tensor_scalar(M[:, t, :], tif[:], float(t), None,
                                    op0=ALU.is_equal)
        w_ = T("w_")
        nc.vector.tensor_scalar(w_[:], u[:], -1.0, 1.0, op0=ALU.mult,
                                op1=ALU.add)
        PR = sb.tile([128, 4, FREE], f16, tag="PR")
        nc.vector.tensor_tensor(PR[:, 0, :], u[:], u2[:], op=ALU.mult)   # u^3
        nc.vector.tensor_tensor(PR[:, 3, :], w_[:], w2[:], op=ALU.mult)  # w^3
        rw = T("rw"); rw2 = T("rw2")
        nc.vector.tensor_scalar(rw[:], w2[:], -2.0, None, op0=ALU.mult)
        nc.vector.tensor_tensor(rw2[:], rw[:], PR[:, 3, :], op=ALU.add)
        nc.vector.tensor_scalar(PR[:, 1, :], rw2[:], 3.0, 4.0, op0=ALU.mult,
                                op1=ALU.add)                     # 3w^3-6w^2+4
        ru = T("ru"); ru2 = T("ru2")
        nc.vector.tensor_scalar(ru[:], u2[:], -2.0, None, op0=ALU.mult)
        nc.vector.tensor_tensor(ru2[:], ru[:], PR[:, 0, :], op=ALU.add)
        nc.vector.tensor_scalar(PR[:, 2, :], ru2[:], 3.0, 4.0, op0=ALU.mult,
                                op1=ALU.add)                     # 3u^3-6u^2+4

        # ---- products GS[t] = m_t (bcast) * PR[0:4]; fused adds for j=2..5
        GS = {}
        for t in range(5):
            GS[t] = sb.tile([128, 4, FREE], f16, tag=f"GS{t}", name=f"GS{t}")
        FS = {j: T(f"FS{j}") for j in (2, 3, 4, 5)}
        aa = T("aa"); bb = T("bb"); aa2 = T("aa2"); bb2 = T("bb2")

        def gs_prod(t):
            mslice = M[:, t, :]
            mb = bass.AP(tensor=mslice.tensor, offset=mslice.offset,
                         ap=[list(mslice.ap[0]), [0, 4], [1, FREE]])
            nc.vector.tensor_tensor(GS[t][:], mb, PR[:], op=ALU.mult)

        # slot of basis j within GS[t]: s = t+3-j
        def gsl(t, j):
            return GS[t][:, t + 3 - j, :]

        gs_prod(0)
        gs_prod(1)
        gs_prod(2)
        # j2 = t0s1 + t1s2 + t2s3
        nc.vector.tensor_tensor(aa[:], gsl(0, 2), gsl(1, 2), op=ALU.add)
        nc.vector.tensor_tensor(FS[2][:], aa[:], gsl(2, 2), op=ALU.add)
        gs_prod(3)
        # j3 = t0s0 + t1s1 + t2s2 + t3s3
        nc.vector.tensor_tensor(bb[:], gsl(0, 3), gsl(1, 3), op=ALU.add)
        nc.vector.tensor_tensor(aa2[:], gsl(2, 3), gsl(3, 3), op=ALU.add)
        nc.vector.tensor_tensor(FS[3][:], bb[:], aa2[:], op=ALU.add)
        gs_prod(4)
        # j4 = t1s0 + t2s1 + t3s2 + t4s3
        nc.vector.tensor_tensor(bb2[:], gsl(1, 4), gsl(2, 4), op=ALU.add)
        nc.vector.tensor_tensor(aa[:], gsl(3, 4), gsl(4, 4), op=ALU.add)
        nc.vector.tensor_tensor(FS[4][:], bb2[:], aa[:], op=ALU.add)
        # j5 = t2s0 + t3s1 + t4s2
        nc.vector.tensor_tensor(bb[:], gsl(2, 5), gsl(3, 5), op=ALU.add)
        nc.vector.tensor_tensor(FS[5][:], bb[:], gsl(4, 5), op=ALU.add)

        # ---- fp8 feature tiles + converts on ACT (completion order) ----
        fq = {}
        for f in range(NFEAT):
            fq[f] = sb.tile([128, CH, BC], f8, tag=f"fq{f}", name=f"fq{f}")

        def conv(f):
            kind, pl = FEATS[f]
            if kind == 'p':
                t, s = pl
                src = GS[t][:, s, :]
            elif kind == 'f':
                src = FS[pl][:]
            else:
                nc.scalar.activation(
                    fq[f][:].rearrange("p c b -> p (c b)"), xr, AF.Silu)
                return
            nc.scalar.activation(fq[f][:].rearrange("p c b -> p (c b)"),
                                 src, AF.Copy)

        # order converts by source availability: GS0 slots, GS1, j2, GS3…
        CONV_ORDER = [0, 1, 2, 3, 7, 4, 8, 9, 5, 6, 10]
        for f in CONV_ORDER:
            conv(f)

        # ---- main matmuls: shared LDWEIGHTS + 2 half matmuls each ----
        psum = [ps.tile([128, D2], f32, tag=f"y{nh}", name=f"y{nh}")
                for nh in range(2)]

        def mm_pair(lhs, rhs_tile, p, start, stop, dr):
            pm = DR if dr else None
            nc.tensor.ldweights(lhs, perf_mode=pm)
            for nh in range(2):
                ifmap = rhs_tile[:, p, :, nh * D2:(nh + 1) * D2] if dr \
                    else rhs_tile[:, nh * D2:(nh + 1) * D2]
                eng = nc.tensor
                ifmap_ap = eng.lower_ap(ifmap.opt({0, 1} if dr else {0}),
                                        opt=False)
                weights_ap = eng.lower_ap(lhs.opt({0, 1} if dr else {0}),
                                          opt=False, for_matmul_weights=True)
                out_ap = eng.lower_ap(psum[nh][:])
                eng.add_instruction(mybir.InstMatmult(
                    name=nc.get_next_instruction_name(),
                    replication_resolution=0, replication_shift_amnt=0,
                    replication_num_rows=0,
                    start_tensor_calc=start, stop_tensor_calc=stop,
                    ins=[ifmap_ap, weights_ap], outs=[out_ap],
                    perf_mode=pm, ldweights=False))

        for f in range(NFEAT):
            j = _feat_j(f)
            for p in range(PAIRS):
                mm_pair(fq[f][:, 2 * p:2 * p + 2, :], wg[j], p,
                        start=(f == 0 and p == 0), stop=False, dr=True)

        # ---- tail: gather 16-row slices of all features, 2 plain matmuls
        gath0 = sb.tile([128, BC], f8, tag="ga0")
        gath1 = sb.tile([48, BC], f8, tag="ga1")
        for f in range(8):
            nc.sync.dma_start(gath0[16 * f:16 * f + 16, :],
                              fq[f][0:PLAST, 6, :])
        for f in range(8, NFEAT):
            nc.sync.dma_start(gath1[16 * (f - 8):16 * (f - 8) + 16, :],
                              fq[f][0:PLAST, 6, :])
        for nh in range(2):
            nc.tensor.matmul(psum[nh][:], gath0[:],
                             wt0[:, nh * D2:(nh + 1) * D2],
                             start=False, stop=False)
            nc.tensor.matmul(psum[nh][:], gath1[:],
                             wt1[:, nh * D2:(nh + 1) * D2],
                             start=False, stop=True)

        # ---- tail: descale, transpose, tanh, layer2, reduce ----
        ysb = sb.tile([128, HO], f32, tag="ysb")
        for nh in range(2):
            nc.vector.tensor_tensor(ysb[:, nh * D2:(nh + 1) * D2],
                                    psum[nh][:],
                                    sbc[:, nh * D2:(nh + 1) * D2],
                                    op=ALU.mult)
        h1t = []
        for k in range(HS):
            pt = ps.tile([128, 128], f32, tag=f"pt{k}", name=f"pt{k}")
            nc.tensor.transpose(pt[:], ysb[:, k * 128:(k + 1) * 128], idt[:])
            st = sb.tile([128, 128], f16, tag=f"h1t{k}", name=f"h1t{k}")
            nc.scalar.activation(st[:], pt[:], AF.Tanh)
            h1t.append(st)
        ps2 = ps.tile([128, D2], f32, tag="ps2")
        for k in range(HS):
            nc.tensor.matmul(ps2[:], h1t[k][:], w1t[:, k, :],
                             start=(k == 0), stop=False)
        nc.tensor.matmul(ps2[:], ones[:], b1r[:], start=False, stop=True)
        h2 = sb.tile([128, D2], f32, tag="h2")
        nc.scalar.activation(h2[:], ps2[:], AF.Tanh)
        prod = sb.tile([128, D2], f32, tag="prod")
        nc.vector.tensor_tensor(prod[:], h2[:], w2b, op=ALU.mult)
        red = sb.tile([128, H], f32, tag="red")
        nc.vector.tensor_reduce(red[:], prod[:].rearrange("p (h d) -> p h d",
                                                          d=32),
                                axis=mybir.AxisListType.X, op=ALU.add)
        lg = sb.tile([128, H], f32, tag="lg")
        nc.vector.tensor_tensor(lg[:], red[:], b2b, op=ALU.add)
        nc.sync.dma_start(out_d, lg[:])

    nc.compile()
    _CACHE["nc"] = nc
    return nc


def _prep_inputs(x, coef, scale_base, scale_sp, lmd, W1, b1, W2, b2):
    import ml_dtypes
    E4 = ml_dtypes.float8_e4m3   # TRN2 fp8e4: IEEE e4m3, max normal 240
    xf = np.asarray(x, np.float64).reshape(B, I)
    coef = np.asarray(coef, np.float64)
    eff = coef * np.asarray(scale_sp, np.float64)[..., None] \
        * np.asarray(lmd, np.float64)[:, :, None, None] / 6.0
    sbl = np.asarray(scale_base, np.float64) \
        * np.asarray(lmd, np.float64)[:, :, None]
    wbig = np.concatenate([eff, sbl[..., None]], -1)           # (H,I,O,9)
    wi = np.ascontiguousarray(wbig.transpose(1, 3, 0, 2)).reshape(I, NF, HO)
    s_col = np.abs(wi).max(axis=(0, 1)) / 240.0 * 1.05         # (640,)
    s_col[s_col == 0] = 1.0
    wq = np.asarray(wi / s_col[None, None, :], E4)             # (I, NF, HO)

    wq8 = wq.view(np.uint8)
    wp = np.empty((NF, 128, PAIRS, 2, HO), np.uint8)
    for p in range(PAIRS):
        for k in range(2):
            blk = wq8[p * 256 + k * 128: p * 256 + (k + 1) * 128]
            wp[:, :, p, k, :] = blk.transpose(1, 0, 2)
    wp = wp.reshape(-1).view(E4)
    # tail rows: feature-slot-major, 16 tail I-rows each
    wt = np.empty((NFEAT * PLAST, HO), np.uint8)
    for f in range(NFEAT):
        j = _feat_j(f)
        wt[f * PLAST:(f + 1) * PLAST, :] = wq8[768:I, j, :]
    wt = wt.view(E4)

    W1 = np.asarray(W1, np.float64)
    w1bd = np.zeros((HO, D2))
    for h in range(H):
        w1bd[h * O:(h + 1) * O, h * 32:(h + 1) * 32] = W1[h]
    c16 = np.ascontiguousarray(
        w1bd.reshape(HS, 128, D2).transpose(1, 0, 2)).astype(
            np.float16).reshape(128, HS * D2)
    b1c = np.asarray(b1, np.float16).reshape(1, D2).copy()
    c32 = np.ascontiguousarray(np.concatenate([
        np.broadcast_to(np.asarray(W2, np.float32).reshape(D2), (128, D2)),
        np.broadcast_to(np.asarray(b2, np.float32).reshape(H), (128, H)),
        np.broadcast_to(s_col.astype(np.float32), (128, HO))],
        1).astype(np.float32))
    idt = np.eye(128, dtype=np.float32)

    in_maps = []
    for core in range(NC):
        xs = xf[core * BC:(core + 1) * BC].T
        xdev = np.zeros((128, CH, BC), np.float32)
        for c in range(CH):
            rows = xs[c * 128:min((c + 1) * 128, I)]
            xdev[0:rows.shape[0], c, :] = rows
        in_maps.append({"x": xdev, "wp": wp, "wt": wt, "c16": c16,
                        "b1": b1c, "c32": c32, "idt": idt})
    return in_maps


def run(inputs, trace=False, tmpdir=None):
    _install_ntff_hook()
    from concourse.bass_utils import run_bass_kernel_spmd
    nc = _build()
    in_maps = _prep_inputs(**inputs)
    res = run_bass_kernel_spmd(nc, in_maps, core_ids=list(range(NC)),
                               trace=trace, tmpdir=tmpdir)
    out = np.concatenate([r["out"] for r in res.results], 0)
    return out.astype(np.float32), res


def kernel(**inputs):
    out, _ = run(inputs)
    return out
